# revision 19
# baseline (speedup 1.0000x reference)
"""Trainium2 Bass kernel: BiDAF-style context-query attention (nn_CQattn).

Reference (per batch b):
    S    = (C@w1)[:,None] + (Q@w2)[None,:] + (C*w3) @ Q.T        # [N, M]
    S1   = softmax_m(S + NEG*Qmask[None,:])
    S2   = softmax_n(S + NEG*Cmask[:,None])
    A    = S1 @ Q
    Bout = S1 @ (S2.T @ C)

v4 design — PE runs ONLY the four essential 512-row matmul groups
(dot3, T, A, Bout = 256 mms/batch); no PE transposes, no tiny mms:

  - dot3 is computed in the TRANSPOSED [m, n] layout (lhsT = Qwt tiles,
    rhs = Ct), so the ACT Exp eviction can bake the per-partition bias
    q2m[m] = q2[m] - 30*Qmask[m]:   P[m, n] = exp(dot3 + q2m[m]).
    P (= e1t tiles) is directly the stationary operand for the A/Bout
    stage (softmax-m weighting baked in; the c1[n] row factor cancels
    in the rowsum normalization).
  - The [n, m] layout (e2b) comes back via a DRAM roundtrip with a
    DMA XBAR transpose read ([M, N] dram -> [128, NT, M] sbuf) —
    bit-exact, zero PE/ACT cost.
  - The S2-softmax row factor c1e[n] = exp(c1[n] - 30*Cmask[n]) is
    folded into the T-stage rhs host-side (Cbs = c1e*C), and into the
    colsum via a DVE multiply against a broadcast c1e row.
  - rowsum1 = DVE free-dim reduce of e2b; colsum2 = DVE mul+reduce of
    e1t * c1eb.  Reciprocals once per batch on [128, NT]/[128, MT].
  - tt eviction scale = q2e/colsum (q2e = exp(q2)(1-Qmask)); A/B
    eviction scale = 1/rowsum.  All scales off the critical PE path.

Sharding: data-parallel over batch: 32 batches / 8 cores = 4 per core.
Self-contained: shapes hardcoded; no sibling imports.

Toolchain note: the walrus build in this container accepts at most one
sem-wait per instruction; _patch_tile_drain_wait_split splits excess
waits onto same-engine NOPs (required for ANY Tile kernel here).
"""

import numpy as np

B, N, M, D = 32, 2048, 512, 512
NCORES = 8
BPC = B // NCORES  # batches per core
NEGC = -30.0  # capped mask bias (see module docstring)

# fp8e4m3 DoubleRow for the dot3 stage (2x PE throughput there).  The
# quantization error enters the outputs only through the softmax logits
# (damped to ~1% rel err); Qwt is pre-scaled by 2^6 so its ~N(0, w3)
# values sit in fp8's normal range, undone by scale=2^-6 inside the Exp.
FP8_DOT3 = True
QWT_SCALE = 64.0

NT = N // 128  # 16 n-tiles
MT = M // 128  # 4 m-tiles
DT = D // 128  # 4 d-tiles
NQ = N // 512  # 4 n-chunks of 512


def _patch_tile_drain_wait_split():
    import concourse.mybir as mybir
    import concourse.tile as tile

    if getattr(tile.TileContext, "_drain_wait_split_patched", False):
        return

    orig_add = tile.TileContext._add_instruction

    def _add_instruction(self, inst):
        si = inst.sync_info
        waits = list(si.on_wait) if si and si.on_wait else []
        if len(waits) > 1 and inst.engine != mybir.EngineType.Unassigned:
            for w in waits[:-1]:
                nop = mybir.InstNoOp(
                    name=self.nc.get_next_instruction_name(), ins=[], outs=[]
                )
                nop.engine = inst.engine
                nop.sync_info = mybir.SyncInfo(on_wait=[w], on_update=[])
                orig_add(self, nop)
            inst.sync_info = mybir.SyncInfo(
                on_wait=[waits[-1]],
                on_update=list(si.on_update) if si.on_update else [],
            )
        orig_add(self, inst)

    tile.TileContext._add_instruction = _add_instruction

    def _drain_and_barrier(self, tick_clock, wait_clock):
        nc = self.nc
        drain_inst = nc.sync.drain()
        wait_clock.add_sem_waits(
            drain_inst.ins, tile.ScopedClock({None: tick_clock.global_clock})
        )
        si = drain_inst.ins.sync_info
        waits = list(si.on_wait) if si and si.on_wait else []
        if len(waits) > 1:
            drain_inst.ins.sync_info = mybir.SyncInfo(
                on_wait=[waits[0]],
                on_update=list(si.on_update) if si and si.on_update else [],
            )
            for w in waits[1:]:
                nop = nc.sync.nop(nofuse=True, hint="drain_wait_split")
                nop.ins.sync_info = mybir.SyncInfo(on_wait=[w], on_update=[])

        nc.all_engine_barrier()
        assert self.sems is not None
        popped = nc._tile_sem_poison_stack.pop()
        assert popped is self._sem_poison
        nc.clear_and_free_semaphores(list(self.sems.allocated().values()))
        nc.all_engine_barrier()

    tile.TileContext._drain_and_barrier = _drain_and_barrier
    tile.TileContext._drain_wait_split_patched = True


def build_nc(n_reps=1):
    import concourse.bass as bass
    import concourse.mybir as mybir
    import concourse.tile as tile

    _patch_tile_drain_wait_split()

    f32 = mybir.dt.float32
    bf16 = mybir.dt.bfloat16
    AF = mybir.ActivationFunctionType
    AX = mybir.AxisListType
    OP = mybir.AluOpType

    fp8 = mybir.dt.float8e4
    ddt = fp8 if FP8_DOT3 else bf16

    nc = bass.Bass()
    Ct_d = nc.dram_tensor("Ct", [BPC, D, N], ddt, kind="ExternalInput")
    Cbs_d = nc.dram_tensor("Cbs", [BPC, N, D], bf16, kind="ExternalInput")
    Qwt_d = nc.dram_tensor("Qwt", [BPC, D, M], ddt, kind="ExternalInput")
    Qb_d = nc.dram_tensor("Qb", [BPC, M, D], bf16, kind="ExternalInput")
    q2m_d = nc.dram_tensor("q2m", [128, BPC, MT], f32, kind="ExternalInput")
    c1eb_d = nc.dram_tensor("c1eb", [BPC, 128, N], bf16, kind="ExternalInput")
    id_d = nc.dram_tensor("identb", [128, 128], bf16, kind="ExternalInput")
    A_d = nc.dram_tensor("A", [BPC, N, D], bf16, kind="ExternalOutput")
    Bo_d = nc.dram_tensor("Bout", [BPC, N, D], bf16, kind="ExternalOutput")

    mm = nc.tensor.matmul

    with tile.TileContext(nc) as tc:
        with (
            tc.tile_pool(name="const", bufs=1) as constp,
            tc.tile_pool(name="ctp", bufs=2) as ctp,
            tc.tile_pool(name="qwtp", bufs=2) as qwtp,
            tc.tile_pool(name="cbsp", bufs=2) as cbsp,
            tc.tile_pool(name="qbp", bufs=2) as qbp,
            tc.tile_pool(name="c1ebp", bufs=2) as c1ebp,
            tc.tile_pool(name="scrp", bufs=2) as scrp,
            tc.tile_pool(name="e1p", bufs=9) as e1p,
            tc.tile_pool(name="e2p", bufs=2) as e2p,
            tc.tile_pool(name="ttp", bufs=8) as ttp,
            tc.tile_pool(name="stp", bufs=4) as stp,
            tc.tile_pool(name="smallp", bufs=16) as smallp,
            tc.tile_pool(name="pf", bufs=5, space="PSUM") as pf,
            tc.tile_pool(name="ptr", bufs=2, space="PSUM") as ptr,
        ):
            q2m = constp.tile([128, BPC, MT], f32, name="q2m")
            nc.sync.dma_start(q2m[:], q2m_d[:])
            identb = constp.tile([128, 128], bf16, name="identb")
            nc.sync.dma_start(identb[:], id_d[:])

            def setup(b):
                # stage inputs (host-prepped layouts); qwt first and ct in
                # n-chunks so dot3t can start after 1/4 of Ct
                qwt = qwtp.tile([128, DT, M], ddt, name="qwt", tag="qwt")
                nc.sync.dma_start(
                    qwt[:], Qwt_d[b].rearrange("(j p) m -> p j m", p=128)
                )
                ct = ctp.tile([128, DT, N], ddt, name="ct", tag="ct")
                for nq in range(NQ):
                    nc.sync.dma_start(
                        ct[:, :, nq * 512 : (nq + 1) * 512],
                        Ct_d[b][:, nq * 512 : (nq + 1) * 512].rearrange(
                            "(j p) n -> p j n", p=128
                        ),
                    )
                cbs = cbsp.tile([128, NT, D], bf16, name="cbs", tag="cbs")
                nc.sync.dma_start(
                    cbs[:], Cbs_d[b].rearrange("(s p) d -> p s d", p=128)
                )
                qb = qbp.tile([128, MT, D], bf16, name="qb", tag="qb")
                nc.sync.dma_start(
                    qb[:], Qb_d[b].rearrange("(u p) d -> p u d", p=128)
                )
                c1eb = c1ebp.tile([128, N], bf16, name="c1eb", tag="c1eb")
                nc.sync.dma_start(c1eb[:], c1eb_d[b])
                cs = smallp.tile([128, MT], f32, name="cs", tag="small")
                return {"b": b, "qwt": qwt, "ct": ct, "cbs": cbs, "qb": qb,
                        "c1eb": c1eb, "cs": cs, "e1t": []}

            def emit_A_chunk(st, i):
                # stage A: P[v] = exp(dot3t + q2m[v])  [128 m, 2048 n]
                b, qwt, ct = st["b"], st["qwt"], st["ct"]
                v, c = divmod(i, NQ)
                if c == 0:
                    e1v = e1p.tile([128, N], bf16, name=f"e1_{v}", tag="e1")
                    st["e1t"].append(e1v)
                e1v = st["e1t"][v]
                pP = pf.tile([128, 512], f32, name="pP", tag="pf")
                if FP8_DOT3:
                    for j in range(0, DT, 2):
                        mm(
                            pP[:],
                            qwt[:, j : j + 2, v * 128 : (v + 1) * 128],
                            ct[:, j : j + 2, c * 512 : (c + 1) * 512],
                            start=(j == 0),
                            stop=(j == DT - 2),
                            perf_mode=mybir.MatmulPerfMode.DoubleRow,
                        )
                else:
                    for j in range(DT):
                        mm(
                            pP[:],
                            qwt[:, j, v * 128 : (v + 1) * 128],
                            ct[:, j, c * 512 : (c + 1) * 512],
                            start=(j == 0),
                            stop=(j == DT - 1),
                        )
                nc.scalar.activation(
                    e1v[:, c * 512 : (c + 1) * 512],
                    pP[:],
                    AF.Exp,
                    bias=q2m[:, st["b"], v : v + 1],
                    scale=(1.0 / QWT_SCALE) if FP8_DOT3 else 1.0,
                )
                if c == NQ - 1:
                    # colsum2[m] for this v: DVE mul+reduce vs broadcast c1e
                    scr = scrp.tile([128, N], bf16, name="scr", tag="scr")
                    nc.vector.tensor_mul(scr[:], e1v[:], st["c1eb"][:])
                    nc.vector.tensor_reduce(
                        st["cs"][:, v : v + 1], scr[:], axis=AX.X, op=OP.add
                    )

            def begin_B(st):
                st["e2b"] = e2p.tile([128, NT, M], bf16, name="e2b", tag="e2b")
                st["rs1"] = smallp.tile([128, NT], f32, name="rs1", tag="small")
                st["pTs"] = [
                    pf.tile([128, D], f32, name=f"pT{u}", tag="pf")
                    for u in range(MT)
                ]

            def emit_B_group(st, t):
                # stage B group t: PE-transpose the 4 [128,128] blocks of P
                # into [n, m] (e2b[:, t, :]), rowsum via ACT accum, and
                # accumulate the T/colsum mms.
                b, e1t, e2b = st["b"], st["e1t"], st["e2b"]
                pt_ = ptr.tile([128, M], bf16, name="pt", tag="ptr")
                for v in range(MT):
                    nc.tensor.transpose(
                        pt_[:, v * 128 : (v + 1) * 128],
                        e1t[v][:, t * 128 : (t + 1) * 128],
                        identb[:],
                    )
                # ACT-only eviction (DVE reads of PSUM contend with PE PSUM
                # writes on real HW); fused rowsum via ACT accum
                nc.scalar.activation(
                    e2b[:, t, :],
                    pt_[:],
                    AF.Copy,
                    accum_out=st["rs1"][:, t : t + 1],
                )
                for u in range(MT):
                    mm(
                        st["pTs"][u][:],
                        e2b[:, t, u * 128 : (u + 1) * 128],
                        st["cbs"][:, t, :],
                        start=(t == 0),
                        stop=(t == NT - 1),
                    )

            def end_B(st):
                r1 = smallp.tile([128, NT], f32, name="r1", tag="small")
                nc.vector.reciprocal(r1[:], st["rs1"][:])
                st["r1"] = r1
                rcall = smallp.tile([128, MT], f32, name="rcall", tag="small")
                nc.vector.reciprocal(rcall[:], st["cs"][:])
                tt = []
                for u in range(MT):
                    ttu = ttp.tile([128, D], bf16, name="tt", tag="tt")
                    nc.scalar.activation(
                        ttu[:], st["pTs"][u][:], AF.Copy,
                        scale=rcall[:, u : u + 1],
                    )
                    tt.append(ttu)
                st["tt"] = tt

            def emit_C(st):
                # stage C: A[t], Bout[t] = diag(1/rowsum1) . P^T @ {Qb, T};
                # evictions on DVE (ACT relief)
                b, e1t, qb, tt, r1 = (
                    st["b"], st["e1t"], st["qb"], st["tt"], st["r1"],
                )
                for g in range(NT // 4):
                    ast = stp.tile([128, 4, D], bf16, name="ast", tag="ast")
                    bst = stp.tile([128, 4, D], bf16, name="bst", tag="bst")
                    for s in range(4):
                        t = g * 4 + s
                        pA = pf.tile([128, D], f32, name="pA", tag="pf")
                        pB = pf.tile([128, D], f32, name="pB", tag="pf")
                        for v in range(MT):
                            lhsT = e1t[v][:, t * 128 : (t + 1) * 128]
                            mm(pA[:], lhsT, qb[:, v, :], start=(v == 0), stop=(v == MT - 1))
                            mm(pB[:], lhsT, tt[v][:], start=(v == 0), stop=(v == MT - 1))
                        nc.vector.tensor_scalar_mul(
                            ast[:, s, :], pA[:], r1[:, t : t + 1]
                        )
                        nc.vector.tensor_scalar_mul(
                            bst[:, s, :], pB[:], r1[:, t : t + 1]
                        )
                    nc.sync.dma_start(
                        A_d[b, g * 512 : (g + 1) * 512, :].rearrange(
                            "(s p) d -> p s d", p=128
                        ),
                        ast[:],
                    )
                    nc.sync.dma_start(
                        Bo_d[b, g * 512 : (g + 1) * 512, :].rearrange(
                            "(s p) d -> p s d", p=128
                        ),
                        bst[:],
                    )

            # fine-grained software pipeline: each stage-A chunk of batch b
            # is interleaved with a stage-B t-group of batch b-1 so the PE
            # stream never waits long on ACT Exp/copy evictions.
            prev = None
            for b in [b for _ in range(n_reps) for b in range(BPC)]:
                st = setup(b)
                if prev is not None:
                    begin_B(prev)
                for i in range(NT):
                    emit_A_chunk(st, i)
                    if prev is not None:
                        emit_B_group(prev, i)
                if prev is not None:
                    end_B(prev)
                    emit_C(prev)
                prev = st
            begin_B(prev)
            for t in range(NT):
                emit_B_group(prev, t)
            end_B(prev)
            emit_C(prev)

    return nc


_NC = None


def _get_nc():
    global _NC
    if _NC is None:
        _NC = build_nc()
        _NC.finalize()
    return _NC


def _make_in_maps(C, Q, Cmask, Qmask, w):
    import ml_dtypes

    bf = ml_dtypes.bfloat16
    C = np.asarray(C, dtype=np.float32)
    Q = np.asarray(Q, dtype=np.float32)
    w = np.asarray(w, dtype=np.float32)
    w1, w2, w3 = w[:D], w[D : 2 * D], w[2 * D :]

    c1 = C @ w1  # [B, N]
    q2 = Q @ w2  # [B, M]
    Cm = np.asarray(Cmask, dtype=np.float32)
    Qm = np.asarray(Qmask, dtype=np.float32)
    c1e = np.exp(c1 + np.float32(NEGC) * Cm).astype(np.float32)  # [B, N]
    q2m_full = (q2 + np.float32(NEGC) * Qm).astype(np.float32)  # [B, M]

    if FP8_DOT3:
        import concourse.mybir as mybir

        f8 = mybir.dt.np(mybir.dt.float8e4)
        Ct = np.ascontiguousarray(C.transpose(0, 2, 1).astype(f8))  # [B, D, N]
        Qwt = np.ascontiguousarray(
            (Q * w3[None, None, :] * np.float32(QWT_SCALE))
            .transpose(0, 2, 1)
            .astype(f8)
        )
    else:
        Ct = np.ascontiguousarray(C.astype(bf).transpose(0, 2, 1))  # [B, D, N]
        Qwt = np.ascontiguousarray(
            (Q * w3[None, None, :]).transpose(0, 2, 1).astype(bf)
        )
    Cbs = np.ascontiguousarray((C * c1e[:, :, None]).astype(bf))  # [B, N, D]
    Qb = np.ascontiguousarray(Q.astype(bf))  # [B, M, D]

    identb = np.eye(128, dtype=np.float32).astype(bf)

    in_maps = []
    for c in range(NCORES):
        bs = slice(c * BPC, (c + 1) * BPC)
        q2m = np.ascontiguousarray(
            q2m_full[bs].reshape(BPC, MT, 128).transpose(2, 0, 1)
        )
        c1eb = np.ascontiguousarray(
            np.broadcast_to(c1e[bs, None, :], (BPC, 128, N)).astype(bf)
        )
        in_maps.append(
            {
                "Ct": np.ascontiguousarray(Ct[bs]),
                "Cbs": np.ascontiguousarray(Cbs[bs]),
                "Qwt": np.ascontiguousarray(Qwt[bs]),
                "Qb": np.ascontiguousarray(Qb[bs]),
                "q2m": q2m,
                "identb": identb,
                "c1eb": c1eb,
            }
        )
    return in_maps


def run_spmd(C, Q, Cmask, Qmask, w, trace=False):
    """Returns ((A, Bout), BassKernelResults)."""
    from concourse.bass_utils import run_bass_kernel_spmd

    nc = _get_nc()
    in_maps = _make_in_maps(C, Q, Cmask, Qmask, w)
    res = run_bass_kernel_spmd(nc, in_maps, list(range(NCORES)), trace=trace)
    A = np.concatenate(
        [np.asarray(r["A"]).astype(np.float32) for r in res.results], axis=0
    )
    Bout = np.concatenate(
        [np.asarray(r["Bout"]).astype(np.float32) for r in res.results], axis=0
    )
    return (A, Bout), res


def kernel(C, Q, Cmask, Qmask, w):
    (A, Bout), _ = run_spmd(C, Q, Cmask, Qmask, w, trace=False)
    return (A, Bout)
